# revision 1
# baseline (speedup 1.0000x reference)
"""Trainium2 Bass kernel for nn_Net_60052232733176 (gnn_message_passing).

Strategy (graph-data parallel, 8 cores):
  - 50 graphs of 1000 nodes; core c handles graph slots [7c, 7c+7) (padded to 56).
  - Host-side sharding re-encodes each graph's edge list as a dense int8
    multiplicity matrix cnt[1024,1024] (pure index preprocessing).
  - On device everything is dense: GAT attention matrices are rank-1
    (e_src[u]+e_dst[v]) masked by cnt; aggregation = PE matmul h^T @ P;
    per-dst softmax denominators via pairwise-tree + transpose + row-reduce;
    self-loops as separate vector terms; TopK pooling via rank counting
    (compare matrix + column sums); readouts via max/sum trees; final MLP
    batched over graphs with log via Newton iterations (no ACT table swap).
  - No gather/scatter on device at all.

Self-contained: hardcodes all shapes; no file reads.
"""
import os
import numpy as np

import concourse.bass as bass
import concourse.bacc as bacc
import concourse.mybir as mybir
import concourse.tile as tile
from concourse.bass_utils import run_bass_kernel_spmd
from concourse.masks import make_identity
from concourse import bass_isa

F32 = mybir.dt.float32
BF16 = mybir.dt.bfloat16
FP16 = mybir.dt.float16
I8 = mybir.dt.int8
AF = mybir.ActivationFunctionType
OP = mybir.AluOpType
AX = mybir.AxisListType

P = 128
B, NPG, D, C = 50, 1000, 128, 10
NP_ = 1024            # padded nodes per graph
NT = NP_ // P         # 8 node tiles
NCORES = 8
G = 7                 # graph slots per core
K1, K2, K3 = 800, 640, 512
BIGM = 100.0          # dead-node fold added to e_src before exp
BIGS = 1.0e30         # dead-node fold for topk scores / readout max

_cache = {}


# ----------------------------------------------------------------------------
# device program
# ----------------------------------------------------------------------------

def _build_program():
    KG = int(os.environ.get("K_GRAPHS", G))
    KMLP = os.environ.get("K_MLP", "1") == "1"
    KTOPK = os.environ.get("K_TOPK", "1") == "1"
    KGAT = int(os.environ.get("K_GAT_LAYERS", 3))
    KGC = int(os.environ.get("K_GC_LAYERS", 3))
    KDBG = os.environ.get("K_DBG", "0") == "1"
    nc = bacc.Bacc(None, target_bir_lowering=False)

    # ---- DRAM tensors ----
    x_d = nc.dram_tensor("x_sh", [G, NP_, D], F32, kind="ExternalInput")
    xT_d = nc.dram_tensor("xT_sh", [G, D, NP_], F32, kind="ExternalInput")
    cnt_d = nc.dram_tensor("cnt_sh", [G, NP_, NP_], I8, kind="ExternalInput")
    m0_d = nc.dram_tensor("m0", [P, NT], F32, kind="ExternalInput")
    wnames = []

    def wparam(name, shape):
        wnames.append(name)
        return nc.dram_tensor(name, shape, F32, kind="ExternalInput")

    Wg_d = [wparam(f"W_g{l}", [D, D]) for l in (1, 2, 3)]
    asd_d = [wparam(f"asd_g{l}", [D, 2]) for l in (1, 2, 3)]
    bg_d = [wparam(f"b_g{l}", [D, 1]) for l in (1, 2, 3)]
    Wr_d = [wparam(f"Wr_c{l}", [D, D]) for l in (1, 2, 3)]
    br_d = [wparam(f"br_c{l}", [D, 1]) for l in (1, 2, 3)]
    Wo_d = [wparam(f"Wo_c{l}", [D, D]) for l in (1, 2, 3)]
    wp_d = {n: wparam(n, [D, 1]) for n in ("w_p20", "w_p30", "w_p11", "w_p21", "w_p31")}
    Wl1_d = wparam("W_l1", [2 * D, D])
    bl1_d = wparam("b_l1", [D, 1])
    Wl2_d = wparam("W_l2", [D, 64])
    bl2_d = wparam("b_l2", [64, 1])
    Wl3_d = wparam("W_l3", [64, C])
    bl3_d = wparam("b_l3", [C, 1])

    out_d = nc.dram_tensor("out", [G, C], F32, kind="ExternalOutput")
    dbg_d = nc.dram_tensor("dbg", [P, NP_], F32, kind="ExternalOutput") if KDBG else None

    with tile.TileContext(nc) as tc:
        import contextlib
        with contextlib.ExitStack() as ctx:
            cp = ctx.enter_context(tc.tile_pool(name="const", bufs=1))
            c8p = ctx.enter_context(tc.tile_pool(name="cnt8", bufs=2))
            cbp = ctx.enter_context(tc.tile_pool(name="cntbf", bufs=2))
            dLp = ctx.enter_context(tc.tile_pool(name="dL", bufs=2))
            dGp = ctx.enter_context(tc.tile_pool(name="dG", bufs=1))
            stp = ctx.enter_context(tc.tile_pool(name="state", bufs=2))
            vp = ctx.enter_context(tc.tile_pool(name="vec", bufs=2))
            psB = ctx.enter_context(tc.tile_pool(name="psB", bufs=2, space="PSUM"))
            psS = ctx.enter_context(tc.tile_pool(name="psS", bufs=2, space="PSUM"))
            psV = ctx.enter_context(tc.tile_pool(name="psV", bufs=1, space="PSUM"))

            # ---- constants ----
            ident = cp.tile([P, P], F32, tag="ident")
            make_identity(nc, ident[:])
            ident_bf = cp.tile([P, P], BF16, tag="identbf")
            nc.vector.tensor_copy(out=ident_bf[:], in_=ident[:])
            ones_bf = cp.tile([P, 1], BF16, tag="onesbf")
            nc.vector.memset(ones_bf[:], 1.0)
            ones_f16 = cp.tile([P, 1], FP16, tag="onesf16")
            nc.vector.memset(ones_f16[:], 1.0)
            # PE warm: absorb gpsimd ident dep
            warm = psS.tile([P, P], F32, tag="t")
            nc.tensor.transpose(out=warm[:], in_=ident[:], identity=ident[:])

            def load_w(dram, shape, tag, dtype=F32):
                t = cp.tile(shape, dtype, tag=tag)
                if dtype == F32:
                    nc.sync.dma_start(out=t[:], in_=dram[:])
                else:
                    tmp = cp.tile(shape, F32, tag=tag + "_st")
                    nc.sync.dma_start(out=tmp[:], in_=dram[:])
                    nc.vector.tensor_copy(out=t[:], in_=tmp[:])
                return t

            def col_to_rep(col_ap, out_tile_slice, dtype_ident=ident):
                """replicate a [128,1] column across partitions into [128,128]."""
                ps = psS.tile([P, P], F32, tag="t")
                nc.tensor.transpose(out=ps[:], in_=col_ap.to_broadcast([P, P]),
                                    identity=ident[:])
                nc.vector.tensor_copy(out=out_tile_slice, in_=ps[:])

            Wg = [load_w(Wg_d[i], [D, D], f"Wg{i}") for i in range(3)]
            asd = [load_w(asd_d[i], [D, 2], f"asd{i}") for i in range(3)]
            bg_col = [load_w(bg_d[i], [D, 1], f"bg{i}") for i in range(3)]
            Wr_bf = [load_w(Wr_d[i], [D, D], f"Wr{i}", BF16) for i in range(3)]
            br_col = [load_w(br_d[i], [D, 1], f"br{i}") for i in range(3)]
            Wo = [load_w(Wo_d[i], [D, D], f"Wo{i}") for i in range(3)]
            wp = {n: load_w(d, [D, 1], n) for n, d in wp_d.items()}
            Wl1a = cp.tile([D, D], F32, tag="Wl1a")
            nc.sync.dma_start(out=Wl1a[:], in_=Wl1_d[0:D, :])
            Wl1b = cp.tile([D, D], F32, tag="Wl1b")
            nc.sync.dma_start(out=Wl1b[:], in_=Wl1_d[D:2 * D, :])
            bl1 = load_w(bl1_d, [D, 1], "bl1")
            Wl2 = load_w(Wl2_d, [D, 64], "Wl2")
            bl2 = load_w(bl2_d, [64, 1], "bl2")
            Wl3 = load_w(Wl3_d, [64, C], "Wl3")
            bl3 = load_w(bl3_d, [C, 1], "bl3")
            m0 = load_w(m0_d, [P, NT], "m0")

            bg_rep = []
            for i in range(3):
                t = cp.tile([P, P], F32, tag=f"bgrep{i}")
                col_to_rep(bg_col[i][:, 0:1], t[:])
                bg_rep.append(t)
            wrep = {}
            for n in wp:
                t = cp.tile([P, P], F32, tag=f"rep_{n}")
                col_to_rep(wp[n][:, 0:1], t[:])
                wrep[n] = t

            # readout accumulators, one pair per graph (keeps graphs' readout
            # accumulation chains independent under tile-granular dep tracking)
            gacc0 = []
            gacc1 = []
            for g in range(G):
                ga = cp.tile([P, 1], F32, tag=f"gacc0_{g}")
                gb = cp.tile([P, 1], F32, tag=f"gacc1_{g}")
                gacc0.append(ga)
                gacc1.append(gb)
            for g in range(G):
                nc.vector.memset(gacc0[g][:], 0.0)
                nc.vector.memset(gacc1[g][:], 0.0)

            CH = [slice(c * P, (c + 1) * P) for c in range(NT)]

            def transpose_pack(src, dst, src_dtype_ident, n=NT):
                """transpose each [128,128] chunk of src into dst (both [128, n*128]).
                All transposes land in one wide PSUM tile; one DVE copy out."""
                ps = psB.tile([P, NP_], F32, tag="pb")
                for c in range(n):
                    nc.tensor.matmul(ps[:, CH[c]], src[:, CH[c]], src_dtype_ident[:],
                                     is_transpose=True)
                nc.scalar.activation(out=dst[:, :n * P], in_=ps[:, :n * P],
                                     func=AF.Copy)

            def colsum_pe(dense, vout, ones_t):
                """vout[p, c] = sum over u of dense[u, v=c*128+p] via PE ones-matmul."""
                row = psV.tile([1, NP_], F32, tag="pvm")
                for h in range(2):
                    for t in range(NT):
                        nc.tensor.matmul(
                            row[:, h * 512:(h + 1) * 512], ones_t[:],
                            dense[:, t * NP_ + h * 512: t * NP_ + (h + 1) * 512],
                            start=(t == 0), stop=(t == NT - 1))
                rowsb = vp.tile([1, NP_], F32, tag="rowsb")
                nc.vector.tensor_copy(out=rowsb[:], in_=row[:])
                psc = psS.tile([P, NT], F32, tag="t")
                for c in range(NT):
                    nc.tensor.matmul(psc[:, c:c + 1], rowsb[0:1, CH[c]],
                                     ident[0:1, 0:1], is_transpose=True)
                nc.vector.tensor_copy(out=vout[:], in_=psc[:])

            def chunk_tree_reduce(src, col_out, op, scale=None):
                """src [128, 1024] node-major packed; col_out [128,1] = reduce over
                nodes; op add or max."""
                t1 = vp.tile([P, 512], F32, tag="rt1")
                nc.vector.tensor_tensor(out=t1[:], in0=src[:, :512], in1=src[:, 512:],
                                        op=op)
                nc.vector.tensor_tensor(out=t1[:, :256], in0=t1[:, :256],
                                        in1=t1[:, 256:], op=op)
                nc.vector.tensor_tensor(out=t1[:, :128], in0=t1[:, :128],
                                        in1=t1[:, 128:256], op=op)
                ps = psS.tile([P, P], F32, tag="t")
                nc.tensor.matmul(ps[:], t1[:, :128], ident[:], is_transpose=True)
                if scale is None:
                    nc.vector.tensor_reduce(out=col_out, in_=ps[:], axis=AX.X, op=op)
                else:
                    tmp = vp.tile([P, 1], F32, tag="rtmp")
                    nc.vector.tensor_reduce(out=tmp[:], in_=ps[:], axis=AX.X, op=op)
                    nc.vector.tensor_scalar(out=col_out, in0=tmp[:], scalar1=scale,
                                            scalar2=None, op0=OP.mult)

            def topk_readout(g, h_next, m_cur, wrep_t, k, need_hT):
                """pool h_next (node-major [128,1024] f32) by top-k of scores.
                Returns (h_pool, m_next, hT_pool or None). Accumulates readout."""
                s = vp.tile([P, NT], F32, tag="s")
                for c in range(NT):
                    prod = vp.tile([P, P], F32, tag="sprod")
                    nc.vector.tensor_tensor(out=prod[:], in0=h_next[:, CH[c]],
                                            in1=wrep_t[:], op=OP.mult)
                    nc.vector.tensor_reduce(out=s[:, c:c + 1], in_=prod[:],
                                            axis=AX.X, op=OP.add)
                # masked scores s' = s + (m-1)*BIGS
                sm = vp.tile([P, NT], F32, tag="sm")
                mf = vp.tile([P, NT], F32, tag="mfs")
                nc.vector.tensor_scalar(out=mf[:], in0=m_cur[:], scalar1=1.0,
                                        scalar2=BIGS, op0=OP.subtract, op1=OP.mult)
                nc.vector.tensor_tensor(out=sm[:], in0=s[:], in1=mf[:], op=OP.add)
                # s_rep [128, 1024] f32
                srep = stp.tile([P, NP_], F32, tag="srep")
                psw4 = psB.tile([P, NP_], F32, tag="pb")
                for c in range(NT):
                    nc.tensor.matmul(psw4[:, CH[c]],
                                     sm[:, c:c + 1].to_broadcast([P, P]), ident[:],
                                     is_transpose=True)
                nc.scalar.activation(out=srep[:], in_=psw4[:], func=AF.Copy)
                # G[j, i] = s'[i] < s'[j], j = t*128+p
                Gm = dGp.tile([P, NT * NP_], FP16, tag="G")
                rrow = psV.tile([1, NP_], F32, tag="pvm")
                for t in range(NT):
                    eng = nc.vector if t % 2 == 0 else nc.gpsimd
                    eng.tensor_scalar(
                        out=Gm[:, t * NP_:(t + 1) * NP_], in0=srep[:],
                        scalar1=sm[:, t:t + 1], scalar2=None, op0=OP.is_lt)
                    for h in range(2):
                        nc.tensor.matmul(
                            rrow[:, h * 512:(h + 1) * 512], ones_f16[:],
                            Gm[:, t * NP_ + h * 512: t * NP_ + (h + 1) * 512],
                            start=(t == 0), stop=(t == NT - 1))
                rank = vp.tile([P, NT], F32, tag="rank")
                rowsb2 = vp.tile([1, NP_], F32, tag="rowsb")
                nc.vector.tensor_copy(out=rowsb2[:], in_=rrow[:])
                psc2 = psS.tile([P, NT], F32, tag="t")
                for c in range(NT):
                    nc.tensor.matmul(psc2[:, c:c + 1], rowsb2[0:1, CH[c]],
                                     ident[0:1, 0:1], is_transpose=True)
                nc.vector.tensor_copy(out=rank[:], in_=psc2[:])
                keep = vp.tile([P, NT], F32, tag="keep")
                nc.vector.tensor_scalar(out=keep[:], in0=rank[:], scalar1=float(k),
                                        scalar2=None, op0=OP.is_lt)
                # pool scale = tanh(s) * keep
                th = vp.tile([P, NT], F32, tag="th")
                nc.scalar.activation(out=th[:], in_=s[:], func=AF.Tanh)
                pool = vp.tile([P, NT], F32, tag="pool")
                nc.vector.tensor_tensor(out=pool[:], in0=th[:], in1=keep[:],
                                        op=OP.mult)
                h_pool = stp.tile([P, NP_], F32, tag="hpool")
                for c in range(NT):
                    nc.vector.tensor_scalar(out=h_pool[:, CH[c]], in0=h_next[:, CH[c]],
                                            scalar1=pool[:, c:c + 1], scalar2=None,
                                            op0=OP.mult)
                # readout: masked max + mean/k
                kf = vp.tile([P, NT], F32, tag="kf")
                nc.vector.tensor_scalar(out=kf[:], in0=keep[:], scalar1=1.0,
                                        scalar2=BIGS, op0=OP.subtract, op1=OP.mult)
                hm = stp.tile([P, NP_], F32, tag="srep")
                for c in range(NT):
                    nc.vector.tensor_scalar(out=hm[:, CH[c]], in0=h_pool[:, CH[c]],
                                            scalar1=kf[:, c:c + 1], scalar2=None,
                                            op0=OP.add)
                mx = vp.tile([P, 1], F32, tag="mx")
                chunk_tree_reduce(hm[:], mx[:], OP.max)
                nc.vector.tensor_tensor(out=gacc0[g][:], in0=gacc0[g][:],
                                        in1=mx[:], op=OP.add)
                mn = vp.tile([P, 1], F32, tag="mn")
                chunk_tree_reduce(h_pool[:], mn[:], OP.add, scale=1.0 / k)
                nc.vector.tensor_tensor(out=gacc1[g][:], in0=gacc1[g][:],
                                        in1=mn[:], op=OP.add)
                hT_pool = None
                if need_hT:
                    hT_pool = stp.tile([P, NP_], F32, tag="hTpool")
                    transpose_pack(h_pool, hT_pool, ident)
                return h_pool, keep, hT_pool

            def gat_layer(li, cnt_bf, hT, m_cur):
                """returns h_next node-major [128,1024] f32 (pre-pool)."""
                # hW node-major bf16 directly: out[u,d'] = sum_d hT[d,u]*W[d,d']
                hW_bf = stp.tile([P, NP_], BF16, tag="hWbf")
                psw = psB.tile([P, NP_], F32, tag="pb")
                for c in range(NT):
                    nc.tensor.matmul(psw[:, CH[c]], hT[:, CH[c]], Wg[li][:],
                                     start=True, stop=True)
                nc.scalar.activation(out=hW_bf[:], in_=psw[:], func=AF.Copy)
                # e vectors: [es | ed] per chunk
                est = vp.tile([P, 2 * NT], F32, tag="est")
                for c in range(NT):
                    pe = psS.tile([P, 2], F32, tag="t")
                    nc.tensor.matmul(pe[:], hT[:, CH[c]], asd[li][:],
                                     start=True, stop=True)
                    nc.vector.tensor_copy(out=est[:, 2 * c:2 * c + 2], in_=pe[:])
                # es2 = es + (m-1)*BIGM ; ed_col strided view
                mf = vp.tile([P, NT], F32, tag="mf")
                nc.vector.tensor_scalar(out=mf[:], in0=m_cur[:], scalar1=1.0,
                                        scalar2=BIGM, op0=OP.subtract, op1=OP.mult)
                es2 = vp.tile([P, NT], F32, tag="es2")
                ed2 = vp.tile([P, NT], F32, tag="ed2")
                for c in range(NT):
                    nc.vector.tensor_tensor(out=es2[:, c:c + 1],
                                            in0=est[:, 2 * c:2 * c + 1],
                                            in1=mf[:, c:c + 1], op=OP.add)
                    nc.vector.tensor_copy(out=ed2[:, c:c + 1],
                                          in_=est[:, 2 * c + 1:2 * c + 2])
                # ed_rep [128, 1024] bf16
                ed_rep = stp.tile([P, NP_], BF16, tag="edrep")
                psw2 = psB.tile([P, NP_], F32, tag="pb")
                for c in range(NT):
                    nc.tensor.matmul(psw2[:, CH[c]],
                                     ed2[:, c:c + 1].to_broadcast([P, P]), ident[:],
                                     is_transpose=True)
                nc.scalar.activation(out=ed_rep[:], in_=psw2[:], func=AF.Copy)
                # P tiles: fused lrelu(ed_rep + es_u) -> exp -> * cnt, per tile,
                # with agg matmuls interleaved so PE starts early
                L = dLp.tile([P, NT * NP_], BF16, tag="L")
                agg_ps = psB.tile([P, NP_], F32, tag="pb")
                drow = psV.tile([1, NP_], F32, tag="pvm")
                for t in range(NT):
                    sl = slice(t * NP_, (t + 1) * NP_)
                    nc.scalar.activation(out=L[:, sl], in_=ed_rep[:], func=AF.Prelu,
                                         alpha=0.2, bias=es2[:, t:t + 1])
                    nc.scalar.activation(out=L[:, sl], in_=L[:, sl], func=AF.Exp)
                    nc.gpsimd.tensor_tensor(out=L[:, sl], in0=L[:, sl],
                                            in1=cnt_bf[:, sl], op=OP.mult)
                    for h in range(2):
                        hs = slice(t * NP_ + h * 512, t * NP_ + (h + 1) * 512)
                        nc.tensor.matmul(
                            agg_ps[:, h * 512:(h + 1) * 512],
                            hW_bf[:, CH[t]], L[:, hs],
                            start=(t == 0), stop=(t == NT - 1))
                        nc.tensor.matmul(
                            drow[:, h * 512:(h + 1) * 512], ones_bf[:], L[:, hs],
                            start=(t == 0), stop=(t == NT - 1))
                # self terms
                lself = vp.tile([P, NT], F32, tag="lself")
                nc.vector.tensor_tensor(out=lself[:], in0=es2[:], in1=ed2[:], op=OP.add)
                nc.scalar.activation(out=lself[:], in_=lself[:], func=AF.Prelu, alpha=0.2)
                nc.scalar.activation(out=lself[:], in_=lself[:], func=AF.Exp)
                # denom (neighbors) per v: finish from drow
                den = vp.tile([P, NT], F32, tag="den")
                drsb = vp.tile([1, NP_], F32, tag="rowsb")
                nc.vector.tensor_copy(out=drsb[:], in_=drow[:])
                pscd = psS.tile([P, NT], F32, tag="t")
                for c in range(NT):
                    nc.tensor.matmul(pscd[:, c:c + 1], drsb[0:1, CH[c]],
                                     ident[0:1, 0:1], is_transpose=True)
                nc.vector.tensor_copy(out=den[:], in_=pscd[:])
                dtot = vp.tile([P, NT], F32, tag="dtot")
                nc.vector.tensor_tensor(out=dtot[:], in0=den[:], in1=lself[:], op=OP.add)
                rd = vp.tile([P, NT], F32, tag="rd")
                nc.vector.reciprocal(out=rd[:], in_=dtot[:])
                csel = vp.tile([P, NT], F32, tag="csel")
                nc.vector.tensor_tensor(out=csel[:], in0=lself[:], in1=rd[:], op=OP.mult)
                outT_sb = stp.tile([P, NP_], F32, tag="outTsb")
                for hh in range(2):
                    nc.scalar.activation(out=outT_sb[:, hh * 512:(hh + 1) * 512],
                                         in_=agg_ps[:, hh * 512:(hh + 1) * 512],
                                         func=AF.Copy)
                # finalize node-major: relu((agg*rd + hW*csel + b) * m)
                h_next = stp.tile([P, NP_], F32, tag="hnext")
                psw3 = psB.tile([P, NP_], F32, tag="pb")
                for c in range(NT):
                    nc.tensor.matmul(psw3[:, CH[c]], outT_sb[:, CH[c]], ident[:],
                                     is_transpose=True)
                for c in range(NT):
                    t1 = vp.tile([P, P], F32, tag="fin1")
                    nc.vector.tensor_scalar(out=t1[:], in0=psw3[:, CH[c]],
                                            scalar1=rd[:, c:c + 1], scalar2=None,
                                            op0=OP.mult)
                    t2 = vp.tile([P, P], F32, tag="fin2")
                    nc.vector.tensor_scalar(out=t2[:], in0=hW_bf[:, CH[c]],
                                            scalar1=csel[:, c:c + 1], scalar2=None,
                                            op0=OP.mult)
                    nc.vector.tensor_tensor(out=t1[:], in0=t1[:], in1=t2[:], op=OP.add)
                    nc.vector.tensor_tensor(out=t1[:], in0=t1[:], in1=bg_rep[li][:],
                                            op=OP.add)
                    nc.vector.tensor_scalar(out=h_next[:, CH[c]], in0=t1[:],
                                            scalar1=m_cur[:, c:c + 1], scalar2=0.0,
                                            op0=OP.mult, op1=OP.max)
                return h_next

            def gc_layer(li, cnt_bf, z_nm, zT, m_cur):
                """GraphConv: relu((lin_rel(sum_src z) + lin_root(z)) * m)."""
                z_bf = stp.tile([P, NP_], BF16, tag="zbf")
                nc.vector.tensor_copy(out=z_bf[:], in_=z_nm[:])
                agg_ps = psB.tile([P, NP_], F32, tag="pb")
                for t in range(NT):
                    for h in range(2):
                        nc.tensor.matmul(
                            agg_ps[:, h * 512:(h + 1) * 512],
                            z_bf[:, CH[t]],
                            cnt_bf[:, t * NP_ + h * 512: t * NP_ + (h + 1) * 512],
                            start=(t == 0), stop=(t == NT - 1))
                aggT_bf = stp.tile([P, NP_], BF16, tag="aggTbf")
                nc.vector.tensor_copy(out=aggT_bf[:], in_=agg_ps[:])
                outT_ps = psB.tile([P, NP_], F32, tag="pb")
                for h in range(2):
                    sl = slice(h * 512, (h + 1) * 512)
                    nc.tensor.matmul(outT_ps[:, sl], Wr_bf[li][:], aggT_bf[:, sl],
                                     start=True, stop=False)
                    nc.tensor.matmul(outT_ps[:, sl], Wo[li][:], zT[:, sl],
                                     start=False, stop=True)
                # + bias in feat-major, -> bf16 for transpose
                outT_sb = stp.tile([P, NP_], F32, tag="outTsb")
                for hh in range(2):
                    nc.vector.tensor_scalar(
                        out=outT_sb[:, hh * 512:(hh + 1) * 512],
                        in0=outT_ps[:, hh * 512:(hh + 1) * 512],
                        scalar1=br_col[li][:, 0:1], scalar2=None, op0=OP.add)
                h_next = stp.tile([P, NP_], F32, tag="hnext")
                psw3 = psB.tile([P, NP_], F32, tag="pb")
                for c in range(NT):
                    nc.tensor.matmul(psw3[:, CH[c]], outT_sb[:, CH[c]], ident[:],
                                     is_transpose=True)
                for c in range(NT):
                    nc.vector.tensor_scalar(out=h_next[:, CH[c]], in0=psw3[:, CH[c]],
                                            scalar1=m_cur[:, c:c + 1], scalar2=0.0,
                                            op0=OP.mult, op1=OP.max)
                return h_next

            # ---- per-graph loop ----
            dbg_src = None
            for g in range(KG):
                # loads
                c8 = c8p.tile([P, NT, NP_], I8, tag="c8")
                for q in range(4):
                    nc.sync.dma_start(
                        out=c8[:, 2 * q:2 * q + 2, :],
                        in_=cnt_d[g].rearrange("(t p) v -> p t v", p=P)[:, 2 * q:2 * q + 2, :])
                cnt_bf = cbp.tile([P, NT * NP_], BF16, tag="cntbf")
                c8f = c8[:].rearrange("p t v -> p (t v)")
                for q in range(4):
                    qs = slice(q * 2 * NP_, (q + 1) * 2 * NP_)
                    nc.gpsimd.tensor_copy(out=cnt_bf[:, qs], in_=c8f[:, qs])
                x_nm = stp.tile([P, NP_], F32, tag="xnm")
                x_nm3 = x_nm[:].rearrange("p (c d) -> p c d", d=D)
                x_in3 = x_d[g].rearrange("(c p) d -> p c d", p=P)
                for q in range(2):
                    nc.sync.dma_start(out=x_nm3[:, q * 4:(q + 1) * 4, :],
                                      in_=x_in3[:, q * 4:(q + 1) * 4, :])
                xT = stp.tile([P, NP_], F32, tag="xT")
                for q in range(2):
                    nc.sync.dma_start(out=xT[:, q * 512:(q + 1) * 512],
                                      in_=xT_d[g][:, q * 512:(q + 1) * 512])

                pools_gat = [("w_p20", K1), ("w_p20", K2), ("w_p30", K3)]
                pools_gc = [("w_p11", K1), ("w_p21", K2), ("w_p31", K3)]

                # interleaved branches: two independent chains for overlap
                hT, m_gat = xT, m0
                z_nm, zT, m_gc = x_nm, xT, m0
                for li in range(3):
                    if li < KGAT:
                        ha = gat_layer(li, cnt_bf, hT, m_gat)
                        dbg_src = ha
                    if li < KGC:
                        hb = gc_layer(li, cnt_bf, z_nm, zT, m_gc)
                        dbg_src = hb
                    if not KTOPK:
                        break
                    if li < KGAT:
                        wn, k = pools_gat[li]
                        _, m_gat, hT = topk_readout(
                            g, ha, m_gat, wrep[wn], k, need_hT=(li < 2))
                    if li < KGC:
                        wn, k = pools_gc[li]
                        z_nm, m_gc, zT = topk_readout(
                            g, hb, m_gc, wrep[wn], k, need_hT=(li < 2))

            if KDBG:
                if dbg_src is not None:
                    nc.sync.dma_start(out=dbg_d[:], in_=dbg_src[:])
                else:
                    zz = vp.tile([P, NP_], F32, tag="zz")
                    nc.vector.memset(zz[:], 0.0)
                    nc.sync.dma_start(out=dbg_d[:], in_=zz[:])
            if not KMLP:
                zo = vp.tile([G, C], F32, tag="zo")
                nc.vector.memset(zo[:], 0.0)
                nc.sync.dma_start(out=out_d[:], in_=zo[:])
            else:
                # ---- MLP over all graphs: logits = W3^T lrelu(W2^T relu(W1^T g + b1) + b2) + b3
                t1_ps = psV.tile([P, G], F32, tag="pvm")
                for g in range(G):
                    nc.tensor.matmul(t1_ps[:, g:g + 1], Wl1a[:], gacc0[g][:],
                                     start=True, stop=False)
                    nc.tensor.matmul(t1_ps[:, g:g + 1], Wl1b[:], gacc1[g][:],
                                     start=False, stop=True)
                t1 = vp.tile([P, G], F32, tag="t1")
                nc.vector.tensor_scalar(out=t1[:], in0=t1_ps[:], scalar1=bl1[:, 0:1],
                                        scalar2=0.0, op0=OP.add, op1=OP.max)
                t2_ps = psV.tile([64, G], F32, tag="pvm")
                nc.tensor.matmul(t2_ps[:], Wl2[:], t1[:], start=True, stop=True)
                t2p = vp.tile([64, G], F32, tag="t2p")
                nc.vector.tensor_scalar(out=t2p[:], in0=t2_ps[:], scalar1=bl2[:, 0:1],
                                        scalar2=None, op0=OP.add)
                t2 = vp.tile([64, G], F32, tag="t2")
                nc.scalar.activation(out=t2[:], in_=t2p[:], func=AF.Prelu, alpha=0.01)
                t3_ps = psV.tile([C, G], F32, tag="pvm")
                nc.tensor.matmul(t3_ps[:], Wl3[:], t2[:], start=True, stop=True)
                lg_cm = vp.tile([C, G], F32, tag="lgcm")
                nc.vector.tensor_scalar(out=lg_cm[:], in0=t3_ps[:], scalar1=bl3[:, 0:1],
                                        scalar2=None, op0=OP.add)
                # transpose -> [G, C]
                lg_ps = psV.tile([G, C], F32, tag="pvm")
                nc.tensor.matmul(lg_ps[:], lg_cm[:], ident[0:C, 0:C], is_transpose=True)
                lg = vp.tile([G, C], F32, tag="lg")
                nc.vector.tensor_copy(out=lg[:], in_=lg_ps[:])
                # log-sum-exp (no max-sub needed; logits are O(1))
                ex = vp.tile([G, C], F32, tag="ex")
                nc.scalar.activation(out=ex[:], in_=lg[:], func=AF.Exp)
                S = vp.tile([G, 1], F32, tag="S")
                nc.vector.tensor_reduce(out=S[:], in_=ex[:], axis=AX.X, op=OP.add)
                # ln(S) via Newton: y += S*exp(-y) - 1   (S in [~3, ~30])
                y = vp.tile([G, 1], F32, tag="y")
                nc.vector.memset(y[:], 2.3)
                for _ in range(6):
                    eny = vp.tile([G, 1], F32, tag="eny")
                    nc.scalar.activation(out=eny[:], in_=y[:], func=AF.Exp, scale=-1.0)
                    nc.vector.tensor_tensor(out=eny[:], in0=eny[:], in1=S[:], op=OP.mult)
                    nc.vector.tensor_scalar(out=eny[:], in0=eny[:], scalar1=1.0,
                                            scalar2=None, op0=OP.subtract)
                    nc.vector.tensor_tensor(out=y[:], in0=y[:], in1=eny[:], op=OP.add)
                outt = vp.tile([G, C], F32, tag="outt")
                nc.vector.tensor_scalar(out=outt[:], in0=lg[:], scalar1=y[:, 0:1],
                                        scalar2=None, op0=OP.subtract)
                nc.sync.dma_start(out=out_d[:], in_=outt[:])

    nc.compile()
    return nc


# ----------------------------------------------------------------------------
# host side
# ----------------------------------------------------------------------------

def _prep_in_maps(inputs):
    x = np.ascontiguousarray(np.asarray(inputs["x"], np.float32))
    ei = np.asarray(inputs["edge_index"]).astype(np.int64)
    src, dst = ei[0], ei[1]
    gid = src // NPG
    sl, dl = src % NPG, dst % NPG

    cnt = np.zeros((B, NP_, NP_), np.int8)
    np.add.at(cnt, (gid, sl, dl), 1)

    x_pad = np.zeros((B, NP_, D), np.float32)
    x_pad[:, :NPG] = x.reshape(B, NPG, D)

    m0 = np.zeros((NP_,), np.float32)
    m0[:NPG] = 1.0
    m0_packed = np.ascontiguousarray(m0.reshape(NT, P).T)  # [P, NT]

    def col(v):
        return np.ascontiguousarray(np.asarray(v, np.float32).reshape(-1, 1))

    weights = {}
    for l in (1, 2, 3):
        weights[f"W_g{l}"] = np.ascontiguousarray(np.asarray(inputs[f"W_g{l}"], np.float32))
        Wg = np.asarray(inputs[f"W_g{l}"], np.float32)
        weights[f"asd_g{l}"] = np.ascontiguousarray(
            Wg @ np.stack([np.asarray(inputs[f"as_g{l}"], np.float32),
                           np.asarray(inputs[f"ad_g{l}"], np.float32)], axis=1))
        weights[f"b_g{l}"] = col(inputs[f"b_g{l}"])
        weights[f"Wr_c{l}"] = np.ascontiguousarray(np.asarray(inputs[f"Wr_c{l}"], np.float32))
        weights[f"br_c{l}"] = col(inputs[f"br_c{l}"])
        weights[f"Wo_c{l}"] = np.ascontiguousarray(np.asarray(inputs[f"Wo_c{l}"], np.float32))
    for n in ("w_p20", "w_p30", "w_p11", "w_p21", "w_p31"):
        w = np.asarray(inputs[n], np.float32)
        weights[n] = col(w / np.linalg.norm(w))
    weights["W_l1"] = np.ascontiguousarray(np.asarray(inputs["W_l1"], np.float32))
    weights["b_l1"] = col(inputs["b_l1"])
    weights["W_l2"] = np.ascontiguousarray(np.asarray(inputs["W_l2"], np.float32))
    weights["b_l2"] = col(inputs["b_l2"])
    weights["W_l3"] = np.ascontiguousarray(np.asarray(inputs["W_l3"], np.float32))
    weights["b_l3"] = col(inputs["b_l3"])

    in_maps = []
    for c in range(NCORES):
        lo = c * G
        hi = min(lo + G, B)
        xs = np.zeros((G, NP_, D), np.float32)
        cs = np.zeros((G, NP_, NP_), np.int8)
        if hi > lo:
            xs[:hi - lo] = x_pad[lo:hi]
            cs[:hi - lo] = cnt[lo:hi]
        xTs = np.ascontiguousarray(xs.transpose(0, 2, 1))
        im = {"x_sh": xs, "xT_sh": xTs, "cnt_sh": cs, "m0": m0_packed}
        im.update(weights)
        in_maps.append(im)
    return in_maps


def kernel(**inputs) -> np.ndarray:
    if "nc" not in _cache:
        _cache["nc"] = _build_program()
    nc = _cache["nc"]
    in_maps = _prep_in_maps(inputs)
    res = run_bass_kernel_spmd(nc, in_maps, list(range(NCORES)))
    out = np.zeros((B, C), np.float32)
    for c in range(NCORES):
        lo = c * G
        hi = min(lo + G, B)
        if hi > lo:
            out[lo:hi] = np.asarray(res.results[c]["out"])[:hi - lo]
    return out



# revision 15
# speedup vs baseline: 1.2758x; 1.2758x over previous
"""Trainium2 Bass kernel for nn_Net_60052232733176 (gnn_message_passing).

Graph-data parallel over 8 cores (7 graphs max per core). Dense per-graph
formulation, feat-major (h^T) primary layout:

  - Host re-encodes edges as a dense bf16 multiplicity matrix cnt[1024,1024]
    with +1 on the diagonal (GAT self-loops); GraphConv subtracts the diag
    term back out with one tensor op.
  - GAT: rank-1 logits exp(lrelu(es_u+ed_v)) masked by cnt; attention
    aggregation and denominators via PE matmuls over a bf16 pairwise tile;
    normalization as a single tensor divide in feat-major space.  Dead
    nodes are handled by a -100 fold on es (exp -> 0) and by pool
    coefficients being exactly 0 (no explicit mask multiplies).
  - TopK pooling: scores via f32r matmuls from the f32 feat-major state;
    rank counting with an fp16 compare tile + ones-matmul column sums;
    pool/mask coefficient rows broadcast to all partitions with gpsimd
    partition_broadcast; readouts are free-axis reduces in feat-major.
  - Final MLP batched over graphs, log-softmax via a Newton iteration for
    ln (keeps everything on the single resident ACT table).

Self-contained: hardcodes all shapes; no file reads.
"""
import os
import numpy as np

import concourse.bass as bass
import concourse.bacc as bacc
import concourse.mybir as mybir
import concourse.tile as tile
from concourse.bass_utils import run_bass_kernel_spmd
from concourse.masks import make_identity

F32 = mybir.dt.float32
F32R = mybir.dt.float32r
BF16 = mybir.dt.bfloat16
FP16 = mybir.dt.float16
AF = mybir.ActivationFunctionType
OP = mybir.AluOpType
AX = mybir.AxisListType

P = 128
B, NPG, D, C = 50, 1000, 128, 10
NP_ = 1024
NT = NP_ // P          # 8 node chunks
NCORES = 8
G = 7                  # graph slots per core
K1, K2, K3 = 800, 640, 512
BIGM = 100.0           # dead-node fold on es before exp
BIGS = 30000.0         # dead-node fold for topk scores (fp16-safe)

_cache = {}

CH = [slice(c * P, (c + 1) * P) for c in range(NT)]


def _build_program():
    KG = int(os.environ.get("K_GRAPHS", G))
    nc = bacc.Bacc(None, target_bir_lowering=False)

    # ---- DRAM tensors ----
    cnt_d = nc.dram_tensor("cnt_sh", [G, P, NT * NP_], BF16, kind="ExternalInput")
    xT_d = nc.dram_tensor("xT_sh", [G, P, NP_], BF16, kind="ExternalInput")
    xnm_d = nc.dram_tensor("xnm_sh", [G, P, NP_], BF16, kind="ExternalInput")
    mfM0_d = nc.dram_tensor("mfM0", [P, NT], F32, kind="ExternalInput")
    mfS0_d = nc.dram_tensor("mfS0", [1, NP_], FP16, kind="ExternalInput")

    def wparam(name, shape, dtype=F32):
        return nc.dram_tensor(name, shape, dtype, kind="ExternalInput")

    Wg_d = [wparam(f"W_g{l}", [D, D], BF16) for l in (1, 2, 3)]
    asd_d = [wparam(f"asd_g{l}", [D, 2], BF16) for l in (1, 2, 3)]
    bg_d = [wparam(f"b_g{l}", [D, 1]) for l in (1, 2, 3)]
    Wr_d = [wparam(f"Wr_c{l}", [D, D], BF16) for l in (1, 2, 3)]
    br_d = [wparam(f"br_c{l}", [D, 1]) for l in (1, 2, 3)]
    Wo_d = [wparam(f"Wo_c{l}", [D, D], BF16) for l in (1, 2, 3)]
    wp_d = {n: wparam(n, [D, 1]) for n in ("w_p20", "w_p30", "w_p11", "w_p21", "w_p31")}
    Wl1_d = wparam("W_l1", [2 * D, D])
    bl1_d = wparam("b_l1", [D, 1])
    Wl2_d = wparam("W_l2", [D, 64])
    bl2_d = wparam("b_l2", [64, 1])
    Wl3_d = wparam("W_l3", [64, C])
    bl3_d = wparam("b_l3", [C, 1])

    out_d = nc.dram_tensor("out", [G, C], F32, kind="ExternalOutput")

    with tile.TileContext(nc) as tc:
        import contextlib
        with contextlib.ExitStack() as ctx:
            cp = ctx.enter_context(tc.tile_pool(name="const", bufs=1))
            dp = ctx.enter_context(tc.tile_pool(name="dma", bufs=2))
            Lp = ctx.enter_context(tc.tile_pool(name="Ltile", bufs=1))
            Gp = ctx.enter_context(tc.tile_pool(name="Gm", bufs=1))
            sp = ctx.enter_context(tc.tile_pool(name="state", bufs=2))
            s3 = ctx.enter_context(tc.tile_pool(name="state3", bufs=3))
            rp = ctx.enter_context(tc.tile_pool(name="reps", bufs=1))
            vp = ctx.enter_context(tc.tile_pool(name="vec", bufs=1))
            psW = ctx.enter_context(tc.tile_pool(name="psW", bufs=1, space="PSUM"))
            psG = ctx.enter_context(tc.tile_pool(name="psG", bufs=1, space="PSUM"))
            psR = ctx.enter_context(tc.tile_pool(name="psR", bufs=1, space="PSUM"))
            psS = ctx.enter_context(tc.tile_pool(name="psS", bufs=1, space="PSUM"))

            # ---- constants ----
            ident = cp.tile([P, P], F32, tag="ident")
            make_identity(nc, ident[:])
            ident_h = cp.tile([P, P], FP16, tag="identh")
            nc.vector.tensor_copy(out=ident_h[:], in_=ident[:])
            ones_bf = cp.tile([P, 1], BF16, tag="onesbf")
            nc.vector.memset(ones_bf[:], 1.0)
            ones_f16 = cp.tile([P, 1], FP16, tag="onesf16")
            nc.vector.memset(ones_f16[:], 1.0)
            # PE warm-up: absorb gpsimd identity dep
            warm = psS.tile([P, 16], F32, tag="pcol")
            nc.tensor.matmul(warm[0:16, :], ident[:, 0:16], ident[:, 0:16],
                             start=True, stop=True)

            def load_w(dram, shape, tag, dtype=F32):
                t = cp.tile(shape, dtype, tag=tag)
                nc.sync.dma_start(out=t[:], in_=dram[:])
                return t

            Wg = [load_w(Wg_d[i], [D, D], f"Wg{i}", BF16) for i in range(3)]
            asd = [load_w(asd_d[i], [D, 2], f"asd{i}", BF16) for i in range(3)]
            bg = [load_w(bg_d[i], [D, 1], f"bg{i}") for i in range(3)]
            Wr = [load_w(Wr_d[i], [D, D], f"Wr{i}", BF16) for i in range(3)]
            br = [load_w(br_d[i], [D, 1], f"br{i}") for i in range(3)]
            Wo = [load_w(Wo_d[i], [D, D], f"Wo{i}", BF16) for i in range(3)]
            wp = {n: load_w(d, [D, 1], n) for n, d in wp_d.items()}
            Wl1a = cp.tile([D, D], F32, tag="Wl1a")
            nc.sync.dma_start(out=Wl1a[:], in_=Wl1_d[0:D, :])
            Wl1b = cp.tile([D, D], F32, tag="Wl1b")
            nc.sync.dma_start(out=Wl1b[:], in_=Wl1_d[D:2 * D, :])
            bl1 = load_w(bl1_d, [D, 1], "bl1")
            Wl2 = load_w(Wl2_d, [D, 64], "Wl2")
            bl2 = load_w(bl2_d, [64, 1], "bl2")
            Wl3 = load_w(Wl3_d, [64, C], "Wl3")
            bl3 = load_w(bl3_d, [C, 1], "bl3")
            mfM0 = load_w(mfM0_d, [P, NT], "mfM0")
            mfS0 = load_w(mfS0_d, [1, NP_], "mfS0", FP16)

            gacc0 = []
            gacc1 = []
            for g in range(G):
                ga = cp.tile([P, 1], F32, tag=f"gacc0_{g}")
                gb = cp.tile([P, 1], F32, tag=f"gacc1_{g}")
                gacc0.append(ga)
                gacc1.append(gb)
            for g in range(G):
                nc.vector.memset(gacc0[g][:], 0.0)
                nc.vector.memset(gacc1[g][:], 0.0)

            pools_gat = [("w_p20", K1), ("w_p20", K2), ("w_p30", K3)]
            pools_gc = [("w_p11", K1), ("w_p21", K2), ("w_p31", K3)]

            def s_mm(out_row, w_col, hT):
                """scores row: out[0,v] = sum_d w[d]*hT[d,v] (f32r moving)."""
                for h in range(2):
                    sl = slice(h * 512, (h + 1) * 512)
                    nc.tensor.matmul(out_row[:, sl], w_col[:], hT[:, sl],
                                     start=True, stop=True)

            def gat_layer(li, cnt_bf, hTb, mfM):
                """input hTb bf16 feat-major; returns hTf_next f32 feat-major."""
                # es/ed columns: est[p, 2c+{0,1}] = {es,ed}[c*128+p]
                est_ps = psS.tile([P, 2 * NT], F32, tag="pcol")
                for c in range(NT):
                    nc.tensor.matmul(est_ps[:, 2 * c:2 * c + 2], hTb[:, CH[c]],
                                     asd[li][:], start=True, stop=True)
                est = vp.tile([P, 2 * NT], F32, tag="est")
                nc.vector.tensor_copy(out=est[:], in_=est_ps[:])
                es2 = vp.tile([P, NT], F32, tag="es2")
                nc.vector.tensor_tensor(
                    out=es2[:], in0=est[:].rearrange("p (c two) -> p c two", two=2)[:, :, 0],
                    in1=mfM[:], op=OP.add)
                # ed_rep[p, v] = ed[v]: per-chunk transpose-broadcast
                edr_ps = psW.tile([P, NP_], F32, tag="gatW")
                edv = est[:].rearrange("p (c two) -> p c two", two=2)
                for c in range(NT):
                    nc.tensor.matmul(edr_ps[:, CH[c]],
                                     edv[:, c, 1:2].to_broadcast([P, P]),
                                     ident[:], is_transpose=True)
                ed_rep = sp.tile([P, NP_], BF16, tag="edrep")
                nc.scalar.activation(out=ed_rep[:], in_=edr_ps[:], func=AF.Copy)
                # hW node-major: hW[p, t*128+d'] = (h W)[t*128+p, d']
                hw_ps = psW.tile([P, NP_], F32, tag="gatW")
                for c in range(NT):
                    nc.tensor.matmul(hw_ps[:, CH[c]], hTb[:, CH[c]], Wg[li][:],
                                     start=True, stop=True)
                hW_bf = sp.tile([P, NP_], BF16, tag="hWbf")
                nc.vector.tensor_copy(out=hW_bf[:], in_=hw_ps[:])
                # pairwise tile: L[p, t*1024+v] = exp(lrelu(es[t*128+p]+ed[v]))*cnt
                L = Lp.tile([P, NT * NP_], BF16, tag="L")
                agg_ps = psW.tile([P, NP_], F32, tag="gatW")
                drow = psR.tile([1, NP_], F32, tag="rows")
                for t in range(NT):
                    sl = slice(t * NP_, (t + 1) * NP_)
                    nc.scalar.activation(out=L[:, sl], in_=ed_rep[:], func=AF.Prelu,
                                         alpha=0.2, bias=es2[:, t:t + 1])
                    nc.scalar.activation(out=L[:, sl], in_=L[:, sl], func=AF.Exp)
                    nc.vector.tensor_tensor(out=L[:, sl], in0=L[:, sl],
                                            in1=cnt_bf[:, sl], op=OP.mult)
                    for h in range(2):
                        hs = slice(t * NP_ + h * 512, t * NP_ + (h + 1) * 512)
                        nc.tensor.matmul(
                            agg_ps[:, h * 512:(h + 1) * 512],
                            hW_bf[:, CH[t]], L[:, hs],
                            start=(t == 0), stop=(t == NT - 1))
                        nc.tensor.matmul(
                            drow[0:1, h * 512:(h + 1) * 512], ones_bf[:], L[:, hs],
                            start=(t == 0), stop=(t == NT - 1))
                # normalize + bias + relu in feat-major
                rd = vp.tile([1, NP_], F32, tag="den")
                nc.vector.reciprocal(out=rd[:], in_=drow[0:1, :])
                rd_rep = rp.tile([P, NP_], F32, tag="denrep")
                nc.gpsimd.partition_broadcast(rd_rep[:], rd[0:1, :])
                hTf = s3.tile([P, NP_], F32, tag="hT")
                nc.vector.tensor_tensor(out=hTf[:], in0=agg_ps[:], in1=rd_rep[:],
                                        op=OP.mult)
                nc.vector.tensor_scalar(out=hTf[:], in0=hTf[:],
                                        scalar1=bg[li][:, 0:1], scalar2=0.0,
                                        op0=OP.add, op1=OP.max)
                return hTf

            def gc_layer(li, cnt_bf, znm_b, zTb, zT_sub):
                """input z node-major bf16 + zT bf16 (+ zT_sub for diag fix);
                returns zTf_next f32 feat-major."""
                agg_ps = psG.tile([P, NP_], F32, tag="gcW")
                for t in range(NT):
                    for h in range(2):
                        nc.tensor.matmul(
                            agg_ps[:, h * 512:(h + 1) * 512],
                            znm_b[:, CH[t]],
                            cnt_bf[:, t * NP_ + h * 512: t * NP_ + (h + 1) * 512],
                            start=(t == 0), stop=(t == NT - 1))
                # subtract the +I diag contribution (GraphConv has no self-loop)
                aggT_bf = sp.tile([P, NP_], BF16, tag="aggTbf")
                nc.vector.tensor_tensor(out=aggT_bf[:], in0=agg_ps[:], in1=zT_sub[:],
                                        op=OP.subtract)
                out_ps = psG.tile([P, NP_], F32, tag="gcW")
                for h in range(2):
                    sl = slice(h * 512, (h + 1) * 512)
                    nc.tensor.matmul(out_ps[:, sl], Wr[li][:], aggT_bf[:, sl],
                                     start=True, stop=False)
                    nc.tensor.matmul(out_ps[:, sl], Wo[li][:], zTb[:, sl],
                                     start=False, stop=True)
                zTf = s3.tile([P, NP_], F32, tag="zT")
                nc.vector.tensor_scalar(out=zTf[:], in0=out_ps[:],
                                        scalar1=br[li][:, 0:1], scalar2=0.0,
                                        op0=OP.add, op1=OP.max)
                return zTf

            def topk_branch(g, li, br_tag, hTf, mfS, w_col, k, out_tile, need_col):
                """topk+readout for one branch. Writes pooled state into
                out_tile; returns (mfS_next, mfM_next_or_None)."""
                s_ps = psR.tile([1, NP_], F32, tag="rows")
                s_mm(s_ps, w_col, hTf)
                sm = vp.tile([1, NP_], FP16, tag=f"sm_{br_tag}")
                nc.vector.tensor_tensor(out=sm[:], in0=s_ps[:], in1=mfS[:],
                                        op=OP.add)
                th = vp.tile([1, NP_], FP16, tag=f"th_{br_tag}")
                nc.scalar.activation(out=th[:], in_=s_ps[:], func=AF.Tanh)
                # masked-score columns (compare scalars): smc[p,c]=sm[c*128+p]
                smc_ps = psS.tile([P, 2 * NT], FP16, tag="pcol2")
                for c in range(NT):
                    nc.tensor.matmul(smc_ps[:, 2 * c:2 * c + 1], sm[0:1, CH[c]],
                                     ident_h[0:1, 0:1], is_transpose=True)
                smc = vp.tile([P, NT], F32, tag=f"smc_{br_tag}")
                nc.vector.tensor_copy(
                    out=smc[:],
                    in_=smc_ps[:].rearrange("p (c two) -> p c two", two=2)[:, :, 0])
                srep = rp.tile([P, NP_], FP16, tag=f"srep_{br_tag}")
                nc.gpsimd.partition_broadcast(srep[:], sm[0:1, :])
                Gm = Gp.tile([P, NT * NP_], FP16, tag=f"Gm_{br_tag}")
                rr = psR.tile([1, NP_], F32, tag="rows")
                for t in range(NT):
                    nc.vector.tensor_scalar(
                        out=Gm[:, t * NP_:(t + 1) * NP_], in0=srep[:],
                        scalar1=smc[:, t:t + 1], scalar2=None, op0=OP.is_lt)
                    for h in range(2):
                        hs = slice(t * NP_ + h * 512, t * NP_ + (h + 1) * 512)
                        nc.tensor.matmul(rr[0:1, h * 512:(h + 1) * 512],
                                         ones_f16[:], Gm[:, hs],
                                         start=(t == 0), stop=(t == NT - 1))
                keep = vp.tile([1, NP_], FP16, tag=f"keep_{br_tag}")
                nc.vector.tensor_scalar(out=keep[:], in0=rr[:], scalar1=float(k),
                                        scalar2=None, op0=OP.is_lt)
                pool = vp.tile([1, NP_], FP16, tag=f"pool_{br_tag}")
                nc.vector.tensor_tensor(out=pool[:], in0=th[:], in1=keep[:],
                                        op=OP.mult)
                mfS_next = vp.tile([1, NP_], FP16, tag=f"mfS_{br_tag}")
                nc.vector.tensor_scalar(out=mfS_next[:], in0=keep[:], scalar1=1.0,
                                        scalar2=BIGS, op0=OP.subtract, op1=OP.mult)
                mfM_next = None
                if need_col:
                    kc_ps = psS.tile([P, 2 * NT], FP16, tag="pcol2")
                    for c in range(NT):
                        nc.tensor.matmul(kc_ps[:, 2 * c:2 * c + 1], keep[0:1, CH[c]],
                                         ident_h[0:1, 0:1], is_transpose=True)
                    mfM_next = vp.tile([P, NT], F32, tag="mfM")
                    nc.vector.tensor_scalar(
                        out=mfM_next[:],
                        in0=kc_ps[:].rearrange("p (c two) -> p c two", two=2)[:, :, 0],
                        scalar1=1.0, scalar2=BIGM, op0=OP.subtract, op1=OP.mult)
                prep = rp.tile([P, NP_], FP16, tag=f"prep_{br_tag}")
                nc.gpsimd.partition_broadcast(prep[:], pool[0:1, :])
                mrep = rp.tile([P, NP_], FP16, tag=f"mrep_{br_tag}")
                nc.gpsimd.partition_broadcast(mrep[:], mfS_next[0:1, :])
                nc.vector.tensor_tensor(out=out_tile[:], in0=hTf[:], in1=prep[:],
                                        op=OP.mult)
                # readout: masked max + mean/k
                hm = rp.tile([P, NP_], F32, tag=f"hm_{br_tag}")
                nc.gpsimd.tensor_tensor(out=hm[:], in0=out_tile[:], in1=mrep[:],
                                        op=OP.add)
                mx = vp.tile([P, 1], F32, tag="mx")
                nc.vector.tensor_reduce(out=mx[:], in_=hm[:], axis=AX.X, op=OP.max)
                nc.vector.tensor_tensor(out=gacc0[g][:], in0=gacc0[g][:],
                                        in1=mx[:], op=OP.add)
                mn = vp.tile([P, 1], F32, tag="mn")
                nc.vector.tensor_reduce(out=mn[:], in_=out_tile[:], axis=AX.X,
                                        op=OP.add)
                nc.vector.tensor_scalar(out=mn[:], in0=mn[:], scalar1=1.0 / k,
                                        scalar2=None, op0=OP.mult)
                nc.vector.tensor_tensor(out=gacc1[g][:], in0=gacc1[g][:],
                                        in1=mn[:], op=OP.add)
                return mfS_next, mfM_next

            # ---- per-graph loop ----
            for g in range(KG):
                cnt_bf = dp.tile([P, NT * NP_], BF16, tag="cnt")
                for q in range(4):
                    qs = slice(q * 2 * NP_, (q + 1) * 2 * NP_)
                    nc.sync.dma_start(out=cnt_bf[:, qs], in_=cnt_d[g][:, qs])
                xT_b = dp.tile([P, NP_], BF16, tag="xT")
                nc.sync.dma_start(out=xT_b[:], in_=xT_d[g][:])
                xnm_b = dp.tile([P, NP_], BF16, tag="xnm")
                nc.sync.dma_start(out=xnm_b[:], in_=xnm_d[g][:])

                hTb, znm_b, zTb = xT_b, xnm_b, xT_b
                zT_sub = xT_b
                mfM = mfM0
                mfS_g = mfS_c = mfS0
                for li in range(3):
                    last = (li == 2)
                    k = pools_gat[li][1]
                    hTf = gat_layer(li, cnt_bf, hTb, mfM)
                    zTf = gc_layer(li, cnt_bf, znm_b, zTb, zT_sub)
                    h_poolT = s3.tile([P, NP_], F32, tag="hT")
                    mfS_g, mfM = topk_branch(
                        g, li, "g", hTf, mfS_g, wp[pools_gat[li][0]], k,
                        h_poolT, need_col=not last)
                    z_poolT = s3.tile([P, NP_], F32, tag="zT")
                    mfS_c, _ = topk_branch(
                        g, li, "c", zTf, mfS_c, wp[pools_gc[li][0]], k,
                        z_poolT, need_col=False)
                    zT_sub = z_poolT
                    if not last:
                        hTb = sp.tile([P, NP_], BF16, tag="hTb")
                        nc.scalar.activation(out=hTb[:], in_=h_poolT[:],
                                             func=AF.Copy)
                        zTb = sp.tile([P, NP_], BF16, tag="zTb")
                        nc.scalar.activation(out=zTb[:], in_=z_poolT[:],
                                             func=AF.Copy)
                        znm_ps = psG.tile([P, NP_], F32, tag="gcW")
                        for c in range(NT):
                            nc.tensor.matmul(znm_ps[:, CH[c]], z_poolT[:, CH[c]],
                                             ident[:], is_transpose=True)
                        znm_b = sp.tile([P, NP_], BF16, tag="znm")
                        nc.vector.tensor_copy(out=znm_b[:], in_=znm_ps[:])

            # ---- MLP over all graphs ----
            t1_ps = psS.tile([P, 16], F32, tag="pcol")
            for g in range(G):
                nc.tensor.matmul(t1_ps[:, g:g + 1], Wl1a[:], gacc0[g][:],
                                 start=True, stop=False)
                nc.tensor.matmul(t1_ps[:, g:g + 1], Wl1b[:], gacc1[g][:],
                                 start=False, stop=True)
            t1 = vp.tile([P, G], F32, tag="t1")
            nc.vector.tensor_scalar(out=t1[:], in0=t1_ps[:, 0:G], scalar1=bl1[:, 0:1],
                                    scalar2=0.0, op0=OP.add, op1=OP.max)
            t2_ps = psS.tile([P, 16], F32, tag="pcol")
            nc.tensor.matmul(t2_ps[0:64, 0:G], Wl2[:], t1[:], start=True, stop=True)
            t2p = vp.tile([64, G], F32, tag="t2p")
            nc.vector.tensor_scalar(out=t2p[:], in0=t2_ps[0:64, 0:G], scalar1=bl2[:, 0:1],
                                    scalar2=None, op0=OP.add)
            t2 = vp.tile([64, G], F32, tag="t2")
            nc.scalar.activation(out=t2[:], in_=t2p[:], func=AF.Prelu, alpha=0.01)
            t3_ps = psS.tile([P, 16], F32, tag="pcol")
            nc.tensor.matmul(t3_ps[0:C, 0:G], Wl3[:], t2[:], start=True, stop=True)
            lg_cm = vp.tile([C, G], F32, tag="lgcm")
            nc.vector.tensor_scalar(out=lg_cm[:], in0=t3_ps[0:C, 0:G], scalar1=bl3[:, 0:1],
                                    scalar2=None, op0=OP.add)
            lg_ps = psS.tile([P, 16], F32, tag="pcol")
            nc.tensor.matmul(lg_ps[0:G, 0:C], lg_cm[:], ident[0:C, 0:C],
                             is_transpose=True)
            lg = vp.tile([G, C], F32, tag="lg")
            nc.vector.tensor_copy(out=lg[:], in_=lg_ps[0:G, 0:C])
            ex = vp.tile([G, C], F32, tag="ex")
            nc.scalar.activation(out=ex[:], in_=lg[:], func=AF.Exp)
            S = vp.tile([G, 1], F32, tag="S")
            nc.vector.tensor_reduce(out=S[:], in_=ex[:], axis=AX.X, op=OP.add)
            # ln(S) via Newton: y += S*exp(-y) - 1
            y = vp.tile([G, 1], F32, tag="y")
            nc.vector.memset(y[:], 2.3)
            for _ in range(6):
                eny = vp.tile([G, 1], F32, tag="eny")
                nc.scalar.activation(out=eny[:], in_=y[:], func=AF.Exp, scale=-1.0)
                nc.vector.tensor_tensor(out=eny[:], in0=eny[:], in1=S[:], op=OP.mult)
                nc.vector.tensor_scalar(out=eny[:], in0=eny[:], scalar1=1.0,
                                        scalar2=None, op0=OP.subtract)
                nc.vector.tensor_tensor(out=y[:], in0=y[:], in1=eny[:], op=OP.add)
            outt = vp.tile([G, C], F32, tag="outt")
            nc.vector.tensor_scalar(out=outt[:], in0=lg[:], scalar1=y[:, 0:1],
                                    scalar2=None, op0=OP.subtract)
            nc.sync.dma_start(out=out_d[:], in_=outt[:])

    nc.compile()
    return nc


# ----------------------------------------------------------------------------
# host side
# ----------------------------------------------------------------------------

def _prep_in_maps(inputs):
    import ml_dtypes
    bf16 = ml_dtypes.bfloat16
    x = np.ascontiguousarray(np.asarray(inputs["x"], np.float32))
    ei = np.asarray(inputs["edge_index"]).astype(np.int64)
    src, dst = ei[0], ei[1]
    gid = src // NPG
    sl, dl = src % NPG, dst % NPG

    cnt = np.zeros((B, NP_, NP_), np.float32)
    np.add.at(cnt, (gid, sl, dl), 1.0)
    idx = np.arange(NP_)
    cnt[:, idx, idx] += 1.0  # GAT self-loops (GraphConv subtracts this back)
    # pack [g, src, dst] -> [g, p, t*1024+dst]
    cnt_pk = np.ascontiguousarray(
        cnt.reshape(B, NT, P, NP_).transpose(0, 2, 1, 3).reshape(B, P, NT * NP_)
    ).astype(bf16)

    x3 = x.reshape(B, NPG, D)
    x_pad = np.zeros((B, NP_, D), np.float32)
    x_pad[:, :NPG] = x3
    xT = np.ascontiguousarray(x_pad.transpose(0, 2, 1)).astype(bf16)  # [B,128,1024]
    xnm = np.ascontiguousarray(
        x_pad.reshape(B, NT, P, D).transpose(0, 2, 1, 3).reshape(B, P, NP_)
    ).astype(bf16)

    m0 = np.zeros((NP_,), np.float32)
    m0[:NPG] = 1.0
    mfM0 = np.ascontiguousarray(((m0 - 1.0) * BIGM).reshape(NT, P).T)  # [128, 8]
    mfS0 = np.ascontiguousarray(((m0 - 1.0) * BIGS).reshape(1, NP_)).astype(np.float16)

    def col(v):
        return np.ascontiguousarray(np.asarray(v, np.float32).reshape(-1, 1))

    weights = {}
    for l in (1, 2, 3):
        Wgl = np.asarray(inputs[f"W_g{l}"], np.float32)
        weights[f"W_g{l}"] = np.ascontiguousarray(Wgl).astype(bf16)
        weights[f"asd_g{l}"] = np.ascontiguousarray(
            Wgl @ np.stack([np.asarray(inputs[f"as_g{l}"], np.float32),
                            np.asarray(inputs[f"ad_g{l}"], np.float32)], axis=1)
        ).astype(bf16)
        weights[f"b_g{l}"] = col(inputs[f"b_g{l}"])
        weights[f"Wr_c{l}"] = np.ascontiguousarray(
            np.asarray(inputs[f"Wr_c{l}"], np.float32)).astype(bf16)
        weights[f"br_c{l}"] = col(inputs[f"br_c{l}"])
        weights[f"Wo_c{l}"] = np.ascontiguousarray(
            np.asarray(inputs[f"Wo_c{l}"], np.float32)).astype(bf16)
    for n in ("w_p20", "w_p30", "w_p11", "w_p21", "w_p31"):
        w = np.asarray(inputs[n], np.float32)
        weights[n] = col(w / np.linalg.norm(w))
    weights["W_l1"] = np.ascontiguousarray(np.asarray(inputs["W_l1"], np.float32))
    weights["b_l1"] = col(inputs["b_l1"])
    weights["W_l2"] = np.ascontiguousarray(np.asarray(inputs["W_l2"], np.float32))
    weights["b_l2"] = col(inputs["b_l2"])
    weights["W_l3"] = np.ascontiguousarray(np.asarray(inputs["W_l3"], np.float32))
    weights["b_l3"] = col(inputs["b_l3"])

    in_maps = []
    for c in range(NCORES):
        lo = c * G
        hi = min(lo + G, B)
        cs = np.zeros((G, P, NT * NP_), bf16)
        xs = np.zeros((G, P, NP_), bf16)
        xn = np.zeros((G, P, NP_), bf16)
        if hi > lo:
            cs[:hi - lo] = cnt_pk[lo:hi]
            xs[:hi - lo] = xT[lo:hi]
            xn[:hi - lo] = xnm[lo:hi]
        im = {"cnt_sh": cs, "xT_sh": xs, "xnm_sh": xn,
              "mfM0": mfM0, "mfS0": mfS0}
        im.update(weights)
        in_maps.append(im)
    return in_maps


def kernel(**inputs) -> np.ndarray:
    if "nc" not in _cache:
        _cache["nc"] = _build_program()
    nc = _cache["nc"]
    in_maps = _prep_in_maps(inputs)
    res = run_bass_kernel_spmd(nc, in_maps, list(range(NCORES)))
    out = np.zeros((B, C), np.float32)
    for c in range(NCORES):
        lo = c * G
        hi = min(lo + G, B)
        if hi > lo:
            out[lo:hi] = np.asarray(res.results[c]["out"])[:hi - lo]
    return out
